# revision 10
# baseline (speedup 1.0000x reference)
import sys

sys.path.insert(0, "/opt/trn_rl_repo")

from contextlib import ExitStack

import numpy as np

import concourse.bass as bass
import concourse.bacc as bacc
import concourse.tile as tile
from concourse import mybir
from concourse.alu_op_type import AluOpType
from concourse.masks import make_identity

B, LQ, LC, D = 64, 256, 2048, 128
NCORES = 8
BPC = B // NCORES

F32 = mybir.dt.float32
U8 = mybir.dt.uint8
EXP = mybir.ActivationFunctionType.Exp
MULT = AluOpType.mult


def build_nc(bpc=BPC, lc=LC, lq=LQ, reps=1):
    nch = lc // 128
    nq = lq // 128

    nc = bacc.Bacc("TRN2", target_bir_lowering=False, debug=False)
    q_d = nc.dram_tensor("query", [bpc, lq, D], F32, kind="ExternalInput").ap()
    c_d = nc.dram_tensor("context", [bpc, lc, D], F32, kind="ExternalInput").ap()
    m_d = nc.dram_tensor("mask", [bpc, lc, lq], U8, kind="ExternalInput").ap()
    qw_d = nc.dram_tensor("query_weights", [D, 1], F32, kind="ExternalInput").ap()
    cw_d = nc.dram_tensor("context_weights", [D, 1], F32, kind="ExternalInput").ap()
    w_d = nc.dram_tensor("dot_weights", [D, D], F32, kind="ExternalInput").ap()
    o_d = nc.dram_tensor("out", [bpc, lc, 4 * D], F32, kind="ExternalOutput").ap()

    with ExitStack() as ctx:
        tc = ctx.enter_context(tile.TileContext(nc))
        const = ctx.enter_context(tc.tile_pool(name="const", bufs=1))
        small = ctx.enter_context(tc.tile_pool(name="small", bufs=3))
        ctxaug_p = ctx.enter_context(tc.tile_pool(name="ctxaug", bufs=2 * nch))
        mask_p = ctx.enter_context(tc.tile_pool(name="maskp", bufs=4))
        eb_p = ctx.enter_context(tc.tile_pool(name="ebp", bufs=4))
        et_p = ctx.enter_context(tc.tile_pool(name="etp", bufs=2 * nq))
        ctxt_p = ctx.enter_context(tc.tile_pool(name="ctxtp", bufs=2))
        rhs_p = ctx.enter_context(tc.tile_pool(name="rhsp", bufs=2 * nq))
        out_p = ctx.enter_context(tc.tile_pool(name="outp", bufs=4))
        ps_tp = ctx.enter_context(tc.tile_pool(name="ps_tp", bufs=2, space="PSUM"))
        ps_mm = ctx.enter_context(tc.tile_pool(name="ps_mm", bufs=2, space="PSUM"))
        ps_m1 = ctx.enter_context(tc.tile_pool(name="ps_m1", bufs=2, space="PSUM"))
        ps_sm = ctx.enter_context(tc.tile_pool(name="ps_sm", bufs=2, space="PSUM"))

        ident = const.tile([128, 128], F32)
        make_identity(nc, ident)
        w_sb = const.tile([128, 128], F32)
        nc.sync.dma_start(out=w_sb, in_=w_d)
        qw_sb = const.tile([128, 1], F32)
        nc.sync.dma_start(out=qw_sb, in_=qw_d)
        cw_sb = const.tile([128, 1], F32)
        nc.sync.dma_start(out=cw_sb, in_=cw_d)
        # WT[e, d] = W[d, e]^T
        wt_ps = ps_sm.tile([128, 128], F32, tag="sm")
        nc.tensor.transpose(wt_ps, w_sb, ident)
        wt_sb = const.tile([128, 128], F32)
        nc.scalar.copy(wt_sb, wt_ps)

        for b in [bb for _ in range(reps) for bb in range(bpc)]:
            # ---- query-side prep ----
            # rhs_big[j] = [query_rows | ones | qTc_rows]  (qTc filled later)
            rhs_big = []
            for j in range(nq):
                rb = rhs_p.tile([128, 257], F32, tag="rhsbig")
                nc.sync.dma_start(out=rb[:, 0:128], in_=q_d[b, j * 128 : (j + 1) * 128, :])
                nc.gpsimd.memset(rb[:, 128:129], 1.0)
                rhs_big.append(rb)
            # qT[d, q]
            qt_sb = small.tile([128, lq], F32, tag="qt")
            for j in range(nq):
                tp = ps_tp.tile([128, 128], F32, tag="tp")
                nc.tensor.transpose(tp, rhs_big[j][:, 0:128], ident)
                nc.scalar.copy(qt_sb[:, j * 128 : (j + 1) * 128], tp)
            # Wq[d, q] = W @ qT, augmented with cw as column lq (-> wcq col lq = ctx_w)
            wq_ps = ps_sm.tile([128, lq], F32, tag="sm")
            nc.tensor.matmul(wq_ps, wt_sb, qt_sb, start=True, stop=True)
            wq_sb = small.tile([128, lq + 1], F32, tag="wq")
            nc.scalar.copy(wq_sb[:, 0:lq], wq_ps)
            nc.gpsimd.tensor_copy(wq_sb[:, lq : lq + 1], cw_sb)
            # g[q] = exp(query @ qw)
            g_sb = small.tile([128, nq], F32, tag="g")
            for j in range(nq):
                g_ps = ps_sm.tile([128, 1], F32, tag="sm")
                nc.tensor.matmul(
                    g_ps,
                    qt_sb[:, j * 128 : (j + 1) * 128],
                    qw_sb,
                    start=True,
                    stop=True,
                )
                nc.scalar.activation(g_sb[:, j : j + 1], g_ps, EXP)

            # ---- context-side prep ----
            ctxt_sb = ctxt_p.tile([128, lc], F32, tag="ctxt")
            ctx_aug = []
            for i in range(nch):
                ca = ctxaug_p.tile([128, 129], F32, tag="ctxaug")
                nc.sync.dma_start(out=ca[:, 0:128], in_=c_d[b, i * 128 : (i + 1) * 128, :])
                nc.gpsimd.memset(ca[:, 128:129], 1.0)
                ctx_aug.append(ca)
                tp = ps_tp.tile([128, 128], F32, tag="tp")
                nc.tensor.transpose(tp, ca[:, 0:128], ident)
                nc.scalar.copy(ctxt_sb[:, i * 128 : (i + 1) * 128], tp)

            # ---- per-chunk: similarity, exp, mask, M1 accumulation, E^T ----
            et_sb = [et_p.tile([128, lc], F32, tag="et", name=f"et{j}") for j in range(nq)]
            m1_ps = [ps_m1.tile([128, 129], F32, tag="m1", name=f"m1_{j}") for j in range(nq)]
            for i in range(nch):
                wcq_ps = ps_mm.tile([128, lq + 1], F32, tag="mm")
                nc.tensor.matmul(
                    wcq_ps,
                    ctxt_sb[:, i * 128 : (i + 1) * 128],
                    wq_sb,
                    start=True,
                    stop=True,
                )
                mk_sb = mask_p.tile([128, lq], U8, tag="mask")
                nc.sync.dma_start(out=mk_sb, in_=m_d[b, i * 128 : (i + 1) * 128, :])
                # ebw = exp([wcq | ctx_w])
                ebw = eb_p.tile([128, lq + 1], F32, tag="ebw")
                nc.scalar.activation(ebw, wcq_ps, EXP)
                # eb = exp(wcq) * exp(ctx_w)[c] * mask
                eb = eb_p.tile([128, lq], F32, tag="eb")
                nc.vector.scalar_tensor_tensor(
                    out=eb,
                    in0=ebw[:, 0:lq],
                    scalar=ebw[:, lq : lq + 1],
                    in1=mk_sb,
                    op0=MULT,
                    op1=MULT,
                )
                for j in range(nq):
                    # M1[q, 0:128] += Eb_chunk.T-contract @ ctx ; M1[q, 128] += colsum(Eb)
                    nc.tensor.matmul(
                        m1_ps[j],
                        eb[:, j * 128 : (j + 1) * 128],
                        ctx_aug[i],
                        start=(i == 0),
                        stop=(i == nch - 1),
                    )
                    tp = ps_tp.tile([128, 128], F32, tag="tp")
                    nc.tensor.transpose(tp, eb[:, j * 128 : (j + 1) * 128], ident)
                    # E^T = g[q] * Eb^T
                    nc.scalar.mul(
                        et_sb[j][:, i * 128 : (i + 1) * 128], tp, g_sb[:, j : j + 1]
                    )

            # ---- q2c attention rows: qTc = M1[:, 0:128] / colsum ----
            h_sb = small.tile([128, nq], F32, tag="h")
            for j in range(nq):
                nc.vector.reciprocal(h_sb[:, j : j + 1], m1_ps[j][:, 128:129])
                nc.vector.tensor_scalar_mul(
                    rhs_big[j][:, 129:257], m1_ps[j][:, 0:128], h_sb[:, j : j + 1]
                )

            # ---- final per-chunk: [ctq | rowsum | out2] = E^T.T-contract @ rhs_big ----
            for i in range(nch):
                fin_ps = ps_mm.tile([128, 257], F32, tag="mm")
                for j in range(nq):
                    nc.tensor.matmul(
                        fin_ps,
                        et_sb[j][:, i * 128 : (i + 1) * 128],
                        rhs_big[j],
                        start=(j == 0),
                        stop=(j == nq - 1),
                    )
                s_sb = small.tile([128, 1], F32, tag="s")
                nc.vector.reciprocal(s_sb, fin_ps[:, 128:129])
                ot = out_p.tile([128, 512], F32, tag="out")
                nc.gpsimd.tensor_copy(ot[:, 0:128], ctx_aug[i][:, 0:128])
                nc.scalar.mul(ot[:, 128:256], fin_ps[:, 0:128], s_sb)
                nc.vector.scalar_tensor_tensor(
                    out=ot[:, 256:384],
                    in0=fin_ps[:, 0:128],
                    scalar=s_sb,
                    in1=ctx_aug[i][:, 0:128],
                    op0=MULT,
                    op1=MULT,
                )
                nc.vector.scalar_tensor_tensor(
                    out=ot[:, 384:512],
                    in0=fin_ps[:, 129:257],
                    scalar=s_sb,
                    in1=ctx_aug[i][:, 0:128],
                    op0=MULT,
                    op1=MULT,
                )
                nc.sync.dma_start(out=o_d[b, i * 128 : (i + 1) * 128, :], in_=ot)
    if not nc.is_finalized():
        nc.finalize()
    return nc


_NC_CACHE = {}


def _get_nc():
    if "nc" not in _NC_CACHE:
        _NC_CACHE["nc"] = build_nc()
    return _NC_CACHE["nc"]


def run(inputs, **spmd_kwargs):
    from concourse.bass_utils import run_bass_kernel_spmd

    nc = _get_nc()
    query = np.ascontiguousarray(np.asarray(inputs["query"], dtype=np.float32))
    context = np.ascontiguousarray(np.asarray(inputs["context"], dtype=np.float32))
    mask = np.ascontiguousarray(np.asarray(inputs["mask"])).view(np.uint8)
    qw = np.ascontiguousarray(np.asarray(inputs["query_weights"], dtype=np.float32))
    cw = np.ascontiguousarray(np.asarray(inputs["context_weights"], dtype=np.float32))
    w = np.ascontiguousarray(np.asarray(inputs["dot_weights"], dtype=np.float32))

    in_maps = []
    for k in range(NCORES):
        sl = slice(k * BPC, (k + 1) * BPC)
        in_maps.append(
            {
                "query": query[sl],
                "context": context[sl],
                "mask": mask[sl],
                "query_weights": qw,
                "context_weights": cw,
                "dot_weights": w,
            }
        )
    return run_bass_kernel_spmd(
        nc, in_maps, core_ids=list(range(NCORES)), **spmd_kwargs
    )


def kernel(**inputs):
    res = run(inputs)
    return np.concatenate([r["out"] for r in res.results], axis=0)
